# revision 29
# baseline (speedup 1.0000x reference)
"""Trainium2 Bass kernel for nn_CrossFusionMamba (2-layer Mamba stack + fusion head).

Self-contained: hardcodes all shapes/sharding. Data-parallel over batch across
8 NeuronCores (8 batch elements per core).

Layout: channels on SBUF partitions, flattened (batch, time) on the free dim
(bt = b*512 + t, 8 batches -> 4096 columns per core).

Optimized v2 vs the first working version:
- The selective scan is DVE-bound (tensor_tensor_scan is ~2 cycles/column and
  cannot run on any other engine), so everything else is pushed off the DVE:
  the depthwise conv and the D*u skip-term run on the PE as diagonal matmuls,
  PSUM evacuations run on ACT, and the whole network (which is column-local
  except the final attention pool) is software-pipelined in half-width (2048
  col) stages so PE/ACT work for stage k+1 executes while the DVE scans stage
  k. B/C rows are re-broadcast per d-block (DMA is far from saturated) so
  PSUM can hold a single full-width y accumulator.
- No DRAM spills for z/dt/dtu; only the 16-row B/C tiles and LN stats take a
  DRAM hop (they need partition-broadcast reads).
"""
import sys

if "/opt/trn_rl_repo" not in sys.path:
    sys.path.insert(0, "/opt/trn_rl_repo")

from contextlib import ExitStack

import numpy as np

import concourse.bacc as bacc
import concourse.tile as tile
import concourse.mybir as mybir
from concourse.bass_utils import run_bass_kernel_spmd

f32 = mybir.dt.float32
bf16 = mybir.dt.bfloat16
AF = mybir.ActivationFunctionType
ALU = mybir.AluOpType
AX = mybir.AxisListType

# model dims
B, L, VD, ID = 64, 512, 64, 32
H, DI, DS, DC, DR, NL = 256, 512, 16, 4, 16, 2
NCORES = 8
BS = B // NCORES          # batches per core (8)
BT = BS * L               # free columns per core (4096)
HT = BT // 2              # stage width (2048 cols = 4 batches)
BH = BS // 2              # batches per stage (4)
HB = H // 128             # 2
DB = DI // 128            # 4
LP = L + DC - 1           # padded per-batch length for conv (515)
POISON = 1.0e9

WEIGHT_NAMES = [
    "vent_in_w", "vent_in_b", "vent_ln_w", "vent_ln_b",
    "m_in_w", "m_conv_w", "m_conv_b", "m_xproj_w", "m_dt_w", "m_dt_b",
    "m_Alog", "m_D", "m_out_w", "m_ln_w", "m_ln_b",
    "pool_w", "pool_b", "img_w1", "img_b1", "img_w2", "img_b2",
    "head_w1", "head_b1", "head_w2", "head_b2",
]


def _build():
    nc = bacc.Bacc("TRN2", target_bir_lowering=False, debug=False)

    # ---- DRAM I/O ----
    xv_d = nc.dram_tensor("xv", [BS, L, VD], f32, kind="ExternalInput")
    xi_d = nc.dram_tensor("xi", [BS, ID], f32, kind="ExternalInput")
    wd = {}
    for name, shape in [
        ("vent_in_w", [H, VD]), ("vent_in_b", [H]), ("vent_ln_w", [H]), ("vent_ln_b", [H]),
        ("m_in_w", [NL, 2 * DI, H]), ("m_conv_w", [NL, DI, DC]), ("m_conv_b", [NL, DI]),
        ("m_xproj_w", [NL, DR + 2 * DS, DI]), ("m_dt_w", [NL, DI, DR]), ("m_dt_b", [NL, DI]),
        ("m_Alog", [NL, DI, DS]), ("m_D", [NL, DI]), ("m_out_w", [NL, H, DI]),
        ("m_ln_w", [NL, H]), ("m_ln_b", [NL, H]),
        ("pool_w", [1, H]), ("pool_b", [1]),
        ("img_w1", [H, ID]), ("img_b1", [H]), ("img_w2", [H, H]), ("img_b2", [H]),
        ("head_w1", [H, 3 * H]), ("head_b1", [H]), ("head_w2", [1, H]), ("head_b2", [1]),
    ]:
        wd[name] = nc.dram_tensor(name, shape, f32, kind="ExternalInput")
    out_d = nc.dram_tensor("out", [1, BS], f32, kind="ExternalOutput")

    # DRAM scratch (ping-pong on stage parity where stages overlap)
    bc_sp = nc.dram_tensor("bc_sp", [2, 2, DS, HT], bf16)    # [par, B/C, n, cols]
    ln32_sp = nc.dram_tensor("ln32_sp", [2, 2, HT], f32)     # [par, mu/msq, cols]
    lnb_sp = nc.dram_tensor("lnb_sp", [2, 2, HT], bf16)      # [par, mu/inv, cols]
    pl32_sp = nc.dram_tensor("pl32_sp", [1, BT], f32)        # pool logits
    plb_sp = nc.dram_tensor("plb_sp", [1, BT], bf16)         # pool attn weights

    with tile.TileContext(nc) as tc, ExitStack() as ctx:
        wpool = ctx.enter_context(tc.tile_pool(name="wpool", bufs=1))
        ap = ctx.enter_context(tc.tile_pool(name="ap", bufs=2))

        # ---------------- constants ----------------
        ident = wpool.tile([128, 128], bf16, name="ident")
        nc.vector.memset(ident[:], 1.0)
        nc.gpsimd.affine_select(ident[:], ident[:], pattern=[[-1, 128]], base=0,
                                channel_multiplier=1, compare_op=ALU.is_equal, fill=0.0)
        ones_col = wpool.tile([128, 1], bf16, name="ones_col")
        nc.vector.memset(ones_col[:], 1.0)
        eps_col = wpool.tile([128, 1], f32, name="eps_col")
        nc.vector.memset(eps_col[:], 1e-5)

        # ---------------- weight preprocessing ----------------
        ld_ctx = ExitStack()
        ldps = ld_ctx.enter_context(tc.tile_pool(name="ldps", bufs=2, space="PSUM"))

        # xv load/transpose first: its DMAs and PE/DVE work overlap the long
        # latency chains of the weight preprocessing below
        xvTs = [ap.tile([VD, HT], bf16, tag="u", bufs=8, name=f"xvT{i}")
                for i in range(2)]
        xv_flat = xv_d.ap().rearrange("b l v -> (b l) v")
        for blk in range(BT // 128):
            nat = ap.tile([128, VD], f32, tag="xvsc", bufs=2, name=f"xvn{blk}")
            nc.sync.dma_start(nat[:], xv_flat[blk * 128:(blk + 1) * 128, :])
            nat16 = ap.tile([128, VD], bf16, tag="xvsc", bufs=2, name=f"xvm{blk}")
            nc.vector.tensor_copy(nat16[:], nat[:])
            tp = ldps.tile([VD, 128], bf16, tag="ldT", name=f"xvp{blk}")
            nc.tensor.transpose(tp[:], nat16[:], ident[:])
            nc.vector.tensor_copy(
                xvTs[blk // 16][:, (blk % 16) * 128:(blk % 16 + 1) * 128], tp[:])

        def load_cols(src_ap, n, name):
            """1-D DRAM vector [n] -> list of [128,1] f32 col tiles."""
            cols = []
            for blk in range((n + 127) // 128):
                m = min(128, n - blk * 128)
                t = wpool.tile([m, 1], f32, name=f"{name}_c{blk}")
                nc.sync.dma_start(t[:, 0:1],
                                  src_ap[blk * 128: blk * 128 + m].rearrange("(a b) -> a b", b=1))
                cols.append(t)
            return cols

        def load_T(src_ap, R, C, name):
            """DRAM [R, C] f32 -> transposed bf16 SBUF tiles: list over C-blocks of [*, R]."""
            nrb = (R + 127) // 128
            ncb = (C + 127) // 128
            outs = []
            for cb in range(ncb):
                cm = min(128, C - cb * 128)
                t = wpool.tile([cm, R], bf16, name=f"{name}_T{cb}")
                outs.append(t)
            for rb in range(nrb):
                rm = min(128, R - rb * 128)
                nat = ap.tile([rm, C], f32, tag="u", bufs=8, name=f"{name}_n{rb}")
                nc.sync.dma_start(nat[:], src_ap[rb * 128: rb * 128 + rm, :])
                nat16 = ap.tile([rm, C], bf16, tag="sz", bufs=4, name=f"{name}_m{rb}")
                nc.vector.tensor_copy(nat16[:], nat[:])
                for cb in range(ncb):
                    cm = min(128, C - cb * 128)
                    tp = ldps.tile([cm, rm], bf16, tag="ldT", name=f"{name}_p{rb}_{cb}")
                    nc.tensor.transpose(tp[:], nat16[:, cb * 128: cb * 128 + cm],
                                        ident[0:rm, 0:rm])
                    nc.vector.tensor_copy(outs[cb][:, rb * 128: rb * 128 + rm], tp[:])
            return outs

        ventT = load_T(wd["vent_in_w"].ap(), H, VD, "ventT")          # 1 x [64, 256]
        vent_b = load_cols(wd["vent_in_b"].ap(), H, "vent_b")
        vlnw = load_cols(wd["vent_ln_w"].ap(), H, "vlnw")
        vlnb = load_cols(wd["vent_ln_b"].ap(), H, "vlnb")
        inwT, xpwT, dtwT, outwT = [], [], [], []
        conv_w, conv_b, dt_b, A_t, D_t, lnw, lnb = [], [], [], [], [], [], []
        for l in range(NL):
            inwT.append(load_T(wd["m_in_w"].ap()[l], 2 * DI, H, f"inwT{l}"))
            xpwT.append(load_T(wd["m_xproj_w"].ap()[l], DR + 2 * DS, DI, f"xpwT{l}"))
            dtwT.append(load_T(wd["m_dt_w"].ap()[l], DI, DR, f"dtwT{l}"))
            outwT.append(load_T(wd["m_out_w"].ap()[l], H, DI, f"outwT{l}"))
            cwl, al = [], []
            for d in range(DB):
                sl = slice(d * 128, (d + 1) * 128)
                cw = wpool.tile([128, DC], f32, name=f"cw{l}_{d}")
                nc.sync.dma_start(cw[:], wd["m_conv_w"].ap()[l, sl, :])
                cwl.append(cw)
                alog = ap.tile([128, DS], f32, tag="sqs", bufs=2, name=f"alog{l}_{d}")
                nc.sync.dma_start(alog[:], wd["m_Alog"].ap()[l, sl, :])
                a = wpool.tile([128, DS], f32, name=f"A{l}_{d}")
                nc.scalar.activation(a[:], alog[:], AF.Exp)
                nc.vector.tensor_scalar_mul(a[:], a[:], -1.0)
                al.append(a)
            conv_w.append(cwl)
            conv_b.append(load_cols(wd["m_conv_b"].ap()[l], DI, f"cb{l}"))
            dt_b.append(load_cols(wd["m_dt_b"].ap()[l], DI, f"dtb{l}"))
            D_t.append(load_cols(wd["m_D"].ap()[l], DI, f"D{l}"))
            A_t.append(al)
            lnw.append(load_cols(wd["m_ln_w"].ap()[l], H, f"lnw{l}"))
            lnb.append(load_cols(wd["m_ln_b"].ap()[l], H, f"lnb{l}"))
        poolT = load_T(wd["pool_w"].ap(), 1, H, "poolT")              # 2 x [128, 1]
        poolb = wpool.tile([1, 1], f32, name="poolb")
        nc.sync.dma_start(poolb[:], wd["pool_b"].ap().rearrange("(a b) -> a b", b=1))
        imgw1T = load_T(wd["img_w1"].ap(), H, ID, "imgw1T")           # 1 x [32, 256]
        imgb1 = load_cols(wd["img_b1"].ap(), H, "imgb1")
        imgw2T = load_T(wd["img_w2"].ap(), H, H, "imgw2T")            # 2 x [128, 256]
        imgb2 = load_cols(wd["img_b2"].ap(), H, "imgb2")
        h1T = load_T(wd["head_w1"].ap(), H, 3 * H, "h1T")             # 6 x [128, 256]
        hb1 = load_cols(wd["head_b1"].ap(), H, "hb1")
        h2T = load_T(wd["head_w2"].ap(), 1, H, "h2T")                 # 2 x [128, 1]
        hb2 = wpool.tile([1, 1], f32, name="hb2")
        nc.sync.dma_start(hb2[:], wd["head_b2"].ap().rearrange("(a b) -> a b", b=1))
        ld_ctx.close()

        # diag(D) stationaries (tiny, persistent); diag(conv_w) tiles are
        # per-layer and rotate through an ap-pool tag (built via closures).
        diag_D = []    # diag_D[l][d]: [128,128] bf16
        for l in range(NL):
            ddl = []
            for d in range(DB):
                t = wpool.tile([128, 128], bf16, name=f"dgD{l}_{d}")
                nc.vector.tensor_scalar_mul(t[:], ident[:], D_t[l][d][:, 0:1])
                ddl.append(t)
            diag_D.append(ddl)
        diag_cw_t = {}  # l -> [d][k] tiles

        # ---------------- pipelined-phase PSUM pool ----------------
        # tags: "y" [128,HT] f32 bufs=1 (4 banks); "pj" [128,512] f32 bufs=2
        # (2 banks); "conv" [128,512] f32 bufs=2 (2 banks) -> exactly 8 banks.
        pps = ctx.enter_context(tc.tile_pool(name="pps", bufs=1, space="PSUM"))

        # ---------------- deferred-emission machinery ----------------
        # Entries are (affinity, fn). Affinity d means: the closure writes a
        # stage tile with d-block index d whose SBUF slot (bufs=5 rotation) is
        # reused from the currently-scanning stage's d-block d-1, so it may
        # only be emitted once phase E of that stage has emitted past d-block
        # d-1 (gate: aff <= current block j). FIFO order is mandatory (RAW
        # links are made at emission time).
        deferred = []

        def defer(fn, aff=-1):
            deferred.append((aff, fn))

        def drain(j, k):
            for _ in range(k):
                if not deferred or deferred[0][0] > j:
                    return
                deferred.pop(0)[1]()

        def drain_all():
            while deferred:
                deferred.pop(0)[1]()

        # ---------------- stage state ----------------
        # x[(l,h2)] -> list of HB tiles [128, HT] (LN output feeding layer l)
        x_t = {}
        u_t = {}     # (l,h2) -> 4 tiles [128,HT]  (post-conv u; later holds y*silu(z))
        sz_t = {}    # silu(z)
        dt_t = {}
        dtu_t = {}
        xo_t = {}    # pre-LN out_proj result per stage

        def emit_ln_stats(xo, par, tag):
            """xo: HB tiles [128,HT] -> DRAM rows mu/inv (bf16) via hop."""
            def stats_chunk(s):
                def fn():
                    sl = slice(s * 512, (s + 1) * 512)
                    ps_x = pps.tile([1, 512], f32, tag="pj", name=f"lnsx_{tag}_{s}")
                    for hb in range(HB):
                        nc.tensor.matmul(ps_x[:], ones_col[:], xo[hb][:, sl],
                                         start=(hb == 0), stop=(hb == HB - 1))
                    sxs = ap.tile([1, 512], f32, tag="lnsl", bufs=1, name=f"lnsxs_{tag}_{s}")
                    nc.scalar.activation(sxs[:], ps_x[:], AF.Identity, scale=1.0 / H)
                    nc.sync.dma_start(ln32_sp.ap()[par, 0, sl].rearrange("(a b) -> a b", a=1), sxs[:])
                    ps_q = pps.tile([1, 512], f32, tag="pj", name=f"lnsq_{tag}_{s}")
                    for hb in range(HB):
                        sq = ap.tile([128, 512], bf16, tag="sqs", bufs=2, name=f"lnq_{tag}_{s}_{hb}")
                        nc.scalar.square(sq[:], xo[hb][:, sl])
                        nc.tensor.matmul(ps_q[:], ones_col[:], sq[:],
                                         start=(hb == 0), stop=(hb == HB - 1))
                    sqs2 = ap.tile([1, 512], f32, tag="lnsl", bufs=1, name=f"lnqs_{tag}_{s}")
                    nc.scalar.activation(sqs2[:], ps_q[:], AF.Identity, scale=1.0 / H)
                    nc.sync.dma_start(ln32_sp.ap()[par, 1, sl].rearrange("(a b) -> a b", a=1), sqs2[:])
                return fn
            for s in range(4):
                defer(stats_chunk(s))

            def rows_fn():
                mu8 = ap.tile([BH, 512], f32, tag="ln8", bufs=3, name=f"mu8_{tag}")
                nc.sync.dma_start(mu8[:], ln32_sp.ap()[par, 0, :].rearrange("(b t) -> b t", b=BH))
                msq8 = ap.tile([BH, 512], f32, tag="ln8", bufs=3, name=f"msq8_{tag}")
                nc.sync.dma_start(msq8[:], ln32_sp.ap()[par, 1, :].rearrange("(b t) -> b t", b=BH))
                var8 = ap.tile([BH, 512], f32, tag="ln8", bufs=3, name=f"var8_{tag}")
                nc.vector.tensor_tensor(var8[:], mu8[:], mu8[:], ALU.mult)
                nc.vector.tensor_tensor(var8[:], msq8[:], var8[:], ALU.subtract)
                sd8 = ap.tile([BH, 512], f32, tag="lnsd", bufs=2, name=f"sd8_{tag}")
                nc.scalar.activation(sd8[:], var8[:], AF.Sqrt, bias=eps_col[0:BH, 0:1])
                inv8 = ap.tile([BH, 512], f32, tag="lnsd", bufs=2, name=f"inv8_{tag}")
                nc.vector.reciprocal(inv8[:], sd8[:])
                mu16 = ap.tile([BH, 512], bf16, tag="ln8h", bufs=2, name=f"mu16_{tag}")
                nc.vector.tensor_copy(mu16[:], mu8[:])
                inv16 = ap.tile([BH, 512], bf16, tag="ln8h", bufs=2, name=f"inv16_{tag}")
                nc.vector.tensor_copy(inv16[:], inv8[:])
                nc.sync.dma_start(lnb_sp.ap()[par, 0, :].rearrange("(b t) -> b t", b=BH), mu16[:])
                nc.sync.dma_start(lnb_sp.ap()[par, 1, :].rearrange("(b t) -> b t", b=BH), inv16[:])
            defer(rows_fn)

        def emit_ln_apply(xo, w_cols, b_cols, par, tag):
            """Returns x tiles [HB][128,HT]; emits deferred apply ops (512-chunked)."""
            x_out = [ap.tile([128, HT], bf16, tag="x", bufs=4, name=f"x_{tag}_{hb}")
                     for hb in range(HB)]

            def apply_chunk(s):
                def fn():
                    sl = slice(s * 512, (s + 1) * 512)
                    mu_rep = ap.tile([128, 512], bf16, tag="lnrep", bufs=2,
                                     name=f"murep_{tag}_{s}")
                    nc.sync.dma_start(mu_rep[:],
                                      lnb_sp.ap()[par, 0, sl].partition_broadcast(128))
                    inv_rep = ap.tile([128, 512], bf16, tag="lnrep", bufs=2,
                                      name=f"invrep_{tag}_{s}")
                    nc.sync.dma_start(inv_rep[:],
                                      lnb_sp.ap()[par, 1, sl].partition_broadcast(128))
                    for hb in range(HB):
                        xc = ap.tile([128, 512], bf16, tag="lnxc", bufs=2,
                                     name=f"xc_{tag}_{hb}_{s}")
                        nc.vector.tensor_tensor(xc[:], xo[hb][:, sl], mu_rep[:],
                                                ALU.subtract)
                        xn = ap.tile([128, 512], bf16, tag="lnxc", bufs=2,
                                     name=f"xn_{tag}_{hb}_{s}")
                        nc.vector.tensor_tensor(xn[:], xc[:], inv_rep[:], ALU.mult)
                        nc.scalar.activation(x_out[hb][:, sl], xn[:], AF.Identity,
                                             scale=w_cols[hb][:, 0:1],
                                             bias=b_cols[hb][:, 0:1])
                return fn
            for s in range(4):
                defer(apply_chunk(s))
            return x_out

        # ---------------- vent input projection (pipeline prologue) ----------------
        if True:
            for h2 in range(2):
                par = h2
                xo0 = []
                for hb in range(HB):
                    xo_v = ap.tile([128, HT], bf16, tag="xo", bufs=2, name=f"vxo{h2}_{hb}")
                    for s in range(4):
                        sl = slice(s * 512, (s + 1) * 512)
                        ps = pps.tile([128, 512], f32, tag="pj", bufs=2,
                                      name=f"vps{h2}_{hb}_{s}")
                        nc.tensor.matmul(ps[:], ventT[0][:, hb * 128:(hb + 1) * 128],
                                         xvTs[h2][:, sl], start=True, stop=True)
                        nc.scalar.activation(xo_v[:, sl], ps[:], AF.Identity,
                                             bias=vent_b[hb][:, 0:1])
                    xo0.append(xo_v)
                emit_ln_stats(xo0, par, f"vent{h2}")
                x_t[(0, h2)] = emit_ln_apply(xo0, vlnw, vlnb, par, f"vent{h2}")
                drain_all()

        # ---------------- per-stage phase emitters (deferred) ----------------
        def push_stage_ACD(l, h2):
            """in_proj(u,z) + conv + xproj + dt_proj for stage (l,h2)."""
            par = h2
            x = x_t[(l, h2)]
            # uraw: per-batch padded layout [128, BH*LP]; 3 zero cols in front of
            # each batch so all conv tap matmuls are full width.
            uraw = [ap.tile([128, BH * LP], bf16, tag="uraw", bufs=2,
                            name=f"uraw{l}{h2}_{d}")
                    for d in range(DB)]
            # u is still read by out_proj (F) closures one stage later, so it
            # needs a full two-stage rotation.
            u = [ap.tile([128, HT], bf16, tag="u", bufs=8, name=f"u{l}{h2}_{d}")
                 for d in range(DB)]
            sz = [ap.tile([128, HT], bf16, tag="sz", bufs=4, name=f"sz{l}{h2}_{d}")
                  for d in range(DB)]
            dtt = [ap.tile([128, HT], bf16, tag="dt", bufs=4, name=f"dt{l}{h2}_{d}")
                   for d in range(DB)]
            dtu = [ap.tile([128, HT], bf16, tag="dtu", bufs=4, name=f"dtu{l}{h2}_{d}")
                   for d in range(DB)]
            u_t[(l, h2)] = u
            sz_t[(l, h2)] = sz
            dt_t[(l, h2)] = dtt
            dtu_t[(l, h2)] = dtu
            xdbl = ap.tile([48, HT], bf16, tag="xdbl", bufs=1, name=f"xdbl{l}{h2}")

            if h2 == 0:
                dcw = [[ap.tile([128, 128], bf16, tag="diag", bufs=20,
                                name=f"dgc{l}_{d}_{k}") for k in range(DC)]
                       for d in range(DB)]
                diag_cw_t[l] = dcw

                def build_diag():
                    for d in range(DB):
                        for k in range(DC):
                            nc.vector.tensor_scalar_mul(dcw[d][k][:], ident[:],
                                                        conv_w[l][d][:, k:k + 1])
                defer(build_diag)

            # in_proj u-half -> uraw (padded), then conv -> silu -> u, per (d, s)
            def a_u(d, s):
                def fn():
                    if s == 0:
                        for bi in range(BH):
                            nc.gpsimd.memset(uraw[d][:, bi * LP: bi * LP + DC - 1], 0.0)
                    ps = pps.tile([128, 512], f32, tag="pj", bufs=2,
                                  name=f"au{l}{h2}_{d}_{s}")
                    for kb in range(HB):
                        nc.tensor.matmul(ps[:], inwT[l][kb][:, d * 128:(d + 1) * 128],
                                         x[kb][:, s * 512:(s + 1) * 512],
                                         start=(kb == 0), stop=(kb == HB - 1))
                    nc.scalar.activation(uraw[d][:, s * LP + DC - 1:(s + 1) * LP],
                                         ps[:], AF.Identity)
                return fn

            def conv_c(d, s):
                def fn():
                    ps = pps.tile([128, 512], f32, tag="conv", bufs=2,
                                  name=f"cv{l}{h2}_{d}_{s}")
                    for k in range(DC):
                        nc.tensor.matmul(ps[:], diag_cw_t[l][d][k][:],
                                         uraw[d][:, s * LP + k: s * LP + k + 512],
                                         start=(k == 0), stop=(k == DC - 1))
                    nc.scalar.activation(u[d][:, s * 512:(s + 1) * 512], ps[:], AF.Silu,
                                         bias=conv_b[l][d][:, 0:1])
                return fn
            for d in range(DB):
                for s in range(4):
                    defer(a_u(d, s))
                for s in range(4):
                    defer(conv_c(d, s), aff=d)

            # in_proj z-half -> silu -> sz
            def a_z(d, s):
                def fn():
                    sl = slice(s * 512, (s + 1) * 512)
                    ps = pps.tile([128, 512], f32, tag="pj", bufs=2,
                                  name=f"az{l}{h2}_{d}_{s}")
                    for kb in range(HB):
                        nc.tensor.matmul(ps[:], inwT[l][kb][:, (4 + d) * 128:(5 + d) * 128],
                                         x[kb][:, sl], start=(kb == 0), stop=(kb == HB - 1))
                    nc.scalar.activation(sz[d][:, sl], ps[:], AF.Silu)
                return fn
            for d in range(DB):
                for s in range(4):
                    defer(a_z(d, s), aff=d + 2)

            # xproj -> xdbl; spill B,C rows
            def c_x(s):
                def fn():
                    sl = slice(s * 512, (s + 1) * 512)
                    ps = pps.tile([48, 512], f32, tag="pj", bufs=2,
                                  name=f"cx{l}{h2}_{s}")
                    for kb in range(DB):
                        nc.tensor.matmul(ps[:], xpwT[l][kb][:, 0:48], u[kb][:, sl],
                                         start=(kb == 0), stop=(kb == DB - 1))
                    nc.scalar.activation(xdbl[:, sl], ps[:], AF.Identity)
                return fn
            for s in range(4):
                defer(c_x(s), aff=3)

            def c_spill():
                nc.sync.dma_start(bc_sp.ap()[par, 0], xdbl[16:32, :])
                nc.sync.dma_start(bc_sp.ap()[par, 1], xdbl[32:48, :])
            defer(c_spill, aff=3)

            # dt_proj -> softplus -> dt (chunk-paired to cut ACT table thrash)
            def d_dt(d, sp):
                def fn():
                    ets = []
                    for s in (2 * sp, 2 * sp + 1):
                        sl = slice(s * 512, (s + 1) * 512)
                        ps = pps.tile([128, 512], f32, tag="pj", bufs=2,
                                      name=f"dt{l}{h2}_{d}_{s}")
                        nc.tensor.matmul(ps[:], dtwT[l][0][0:16, d * 128:(d + 1) * 128],
                                         xdbl[0:16, sl], start=True, stop=True)
                        # softplus(x+b) = ln(1 + exp(x+b))
                        et = ap.tile([128, 512], bf16, tag="sqs", bufs=2,
                                     name=f"et{l}{h2}_{d}_{s}")
                        nc.scalar.activation(et[:], ps[:], AF.Exp, bias=dt_b[l][d][:, 0:1])
                        ets.append((s, et))
                    for s, et in ets:
                        nc.scalar.activation(dtt[d][:, s * 512:(s + 1) * 512], et[:],
                                             AF.Ln, bias=1.0)
                return fn

            # dtu = dt*u; then poison dt at batch starts
            def d_fin(d):
                def fn():
                    nc.vector.tensor_tensor(dtu[d][:], dtt[d][:], u[d][:], ALU.mult)
                    for bi in range(BH):
                        nc.gpsimd.memset(dtt[d][:, bi * L: bi * L + 1], POISON)
                return fn
            for d in range(DB):
                for sp in range(2):
                    defer(d_dt(d, sp), aff=max(d + 1, 3))
                defer(d_fin(d), aff=max(d + 1, 3))

        rep_prime = {}

        def emit_stage_E(l, h2, prime_par=None):
            """Selective scan for stage (l,h2). Inline emission; drains deferred.

            dA exps are emitted in adjacent pairs (one ACT table load per
            pair); deferred closures drain in bursts every 4th n; the PSUM-
            gated yz multiply for block d is deferred into block d+1 so the
            DVE never waits on the PE matmul drain at a block boundary.
            """
            u = u_t[(l, h2)]
            sz = sz_t[(l, h2)]
            dtt = dt_t[(l, h2)]
            dtu = dtu_t[(l, h2)]
            par = h2
            pending_yz = [None]

            def flush_yz():
                if pending_yz[0] is not None:
                    pending_yz[0]()
                    pending_yz[0] = None

            def load_rep(d, n):
                repB = ap.tile([128, HT], bf16, tag="rep", bufs=2,
                               name=f"rb{l}{h2}_{d}_{n}")
                nc.sync.dma_start(repB[:],
                                  bc_sp.ap()[par, 0, n, :].partition_broadcast(128))
                repC = ap.tile([128, HT], bf16, tag="rep", bufs=2,
                               name=f"rc{l}{h2}_{d}_{n}")
                nc.sync.dma_start(repC[:],
                                  bc_sp.ap()[par, 1, n, :].partition_broadcast(128))
                return repB, repC

            rep_next = rep_prime.pop(par, None)
            if rep_next is None:
                rep_next = load_rep(0, 0)
            for d in range(DB):
                y_ps = pps.tile([128, HT], f32, tag="y", name=f"yps{l}{h2}_{d}")
                dA_tiles = {}
                hc_bl = []
                for n in range(DS):
                    repB, repC = rep_next
                    rep_next = load_rep(d, n + 1) if n < DS - 1 else (
                        load_rep(d + 1, 0) if d < DB - 1 else None)
                    if n % 2 == 0:
                        for nn in (n, n + 1):
                            dA = ap.tile([128, HT], bf16, tag="pa", bufs=2,
                                         name=f"dA{l}{h2}_{d}_{nn}")
                            nc.scalar.activation(dA[:], dtt[d][:], AF.Exp,
                                                 scale=A_t[l][d][:, nn:nn + 1])
                            dA_tiles[nn] = dA
                    dBu = ap.tile([128, HT], bf16, tag="pb", bufs=1,
                                  name=f"dBu{l}{h2}_{d}_{n}")
                    nc.vector.tensor_tensor(dBu[:], dtu[d][:], repB[:], ALU.mult)
                    hh = ap.tile([128, HT], bf16, tag="ph", bufs=1,
                                 name=f"h{l}{h2}_{d}_{n}")
                    nc.vector.tensor_tensor_scan(hh[:], dA_tiles.pop(n)[:], dBu[:], 0.0,
                                                 ALU.mult, ALU.add)
                    hc = ap.tile([128, HT], bf16, tag="pc", bufs=3,
                                 name=f"hc{l}{h2}_{d}_{n}")
                    nc.vector.tensor_tensor(hc[:], hh[:], repC[:], ALU.mult)
                    # block d's y-matmuls reuse the PSUM banks of block d-1:
                    # buffer hc for n<2, flush the deferred yz(d-1), then emit
                    # the backlog.
                    if n < 2:
                        hc_bl.append(hc)
                        if n == 1:
                            flush_yz()
                            for bi, hcb in enumerate(hc_bl):
                                for si in range(4):
                                    sl = slice(si * 512, (si + 1) * 512)
                                    nc.tensor.matmul(y_ps[:, sl], ident[:], hcb[:, sl],
                                                     start=(bi == 0), stop=False)
                    else:
                        for si in range(4):
                            sl = slice(si * 512, (si + 1) * 512)
                            nc.tensor.matmul(y_ps[:, sl], ident[:], hc[:, sl],
                                             start=False, stop=False)
                    if n % 4 == 3 and n < 15:
                        drain(d, 10)
                # D*u skip-term into the same accumulation
                for si in range(4):
                    sl = slice(si * 512, (si + 1) * 512)
                    nc.tensor.matmul(y_ps[:, sl], diag_D[l][d][:], u[d][:, sl],
                                     start=False, stop=True)

                def yz_fn(d=d, y_ps=y_ps):
                    # y * silu(z) -> overwrite u[d] (dead after the diag_D matmul)
                    nc.vector.tensor_tensor(u[d][:], y_ps[:], sz[d][:], ALU.mult)
                pending_yz[0] = yz_fn
            flush_yz()
            if prime_par is not None:
                # make sure the next stage's B/C spill is emitted, then start
                # its first broadcast so the next E phase doesn't wait on DMA
                drain(3, 999)
                sp = par
                par = prime_par
                rep_prime[prime_par] = load_rep(0, 0)
                par = sp

        def push_stage_F(l, h2):
            """out_proj + LN -> x for (l+1, h2) (or final x)."""
            par = h2
            u = u_t[(l, h2)]   # holds y*silu(z)
            xo = [ap.tile([128, HT], bf16, tag="xo", bufs=2, name=f"xo{l}{h2}_{hb}")
                  for hb in range(HB)]
            xo_t[(l, h2)] = xo

            def f_o(hb, s):
                def fn():
                    sl = slice(s * 512, (s + 1) * 512)
                    ps = pps.tile([128, 512], f32, tag="pj", name=f"fo{l}{h2}_{hb}_{s}")
                    for kb in range(DB):
                        nc.tensor.matmul(ps[:], outwT[l][kb][:, hb * 128:(hb + 1) * 128],
                                         u[kb][:, sl], start=(kb == 0), stop=(kb == DB - 1))
                    nc.scalar.activation(xo[hb][:, sl], ps[:], AF.Identity)
                return fn
            for hb in range(HB):
                for s in range(4):
                    defer(f_o(hb, s))
            emit_ln_stats(xo, par, f"l{l}{h2}")
            x_t[(l + 1, h2)] = emit_ln_apply(xo, lnw[l], lnb[l], par, f"l{l}{h2}")

        # ---------------- the pipeline ----------------
        push_stage_ACD(0, 0)
        drain_all()
        push_stage_ACD(0, 1)
        emit_stage_E(0, 0)
        drain_all()
        push_stage_F(0, 0)
        push_stage_ACD(1, 0)
        emit_stage_E(0, 1)
        drain_all()
        push_stage_F(0, 1)
        push_stage_ACD(1, 1)
        emit_stage_E(1, 0)
        drain_all()
        push_stage_F(1, 0)
        emit_stage_E(1, 1)
        drain_all()
        push_stage_F(1, 1)
        drain_all()

        # ---------------- attention pool over time ----------------
        xf = {h2: x_t[(NL, h2)] for h2 in range(2)}
        for h2 in range(2):
            for s in range(4):
                sl = slice(s * 512, (s + 1) * 512)
                gsl = slice(h2 * HT + s * 512, h2 * HT + (s + 1) * 512)
                ps = pps.tile([1, 512], f32, tag="pj", name=f"pps{h2}_{s}")
                for hb in range(HB):
                    nc.tensor.matmul(ps[:], poolT[hb][:, 0:1], xf[h2][hb][:, sl],
                                     start=(hb == 0), stop=(hb == HB - 1))
                lgs = ap.tile([1, 512], f32, tag="lnsl", bufs=1, name=f"lgs{h2}_{s}")
                nc.scalar.activation(lgs[:], ps[:], AF.Identity, bias=poolb[0:1, 0:1])
                nc.sync.dma_start(pl32_sp.ap()[0:1, gsl], lgs[:])
        lgp = ap.tile([BS, L], f32, tag="lgp", bufs=1, name="lgp")
        nc.sync.dma_start(lgp[:], pl32_sp.ap()[0, :].rearrange("(b t) -> b t", b=BS))
        mx = ap.tile([BS, 1], f32, tag="smc", bufs=4, name="mx")
        nc.vector.tensor_reduce(mx[:], lgp[:], axis=AX.X, op=ALU.max)
        nmx = ap.tile([BS, 1], f32, tag="smc", bufs=4, name="nmx")
        nc.vector.tensor_scalar_mul(nmx[:], mx[:], -1.0)
        ex = ap.tile([BS, L], f32, tag="lgp2", bufs=1, name="ex")
        nc.scalar.activation(ex[:], lgp[:], AF.Exp, bias=nmx[:, 0:1])
        sm = ap.tile([BS, 1], f32, tag="smc", bufs=4, name="sm")
        nc.vector.tensor_reduce(sm[:], ex[:], axis=AX.X, op=ALU.add)
        rs = ap.tile([BS, 1], f32, tag="smc", bufs=4, name="rs")
        nc.vector.reciprocal(rs[:], sm[:])
        aw = ap.tile([BS, L], bf16, tag="aw", bufs=1, name="aw")
        nc.vector.tensor_scalar_mul(aw[:], ex[:], rs[:, 0:1])
        nc.sync.dma_start(plb_sp.ap()[0, :].rearrange("(b t) -> b t", b=BS), aw[:])
        v_t = []
        for hb in range(HB):
            vv = ap.tile([128, BS], f32, tag="vsm", bufs=2, name=f"vv{hb}")
            for h2 in range(2):
                hsl = slice(h2 * HT, (h2 + 1) * HT)
                a_rep = ap.tile([128, HT], bf16, tag="rep", bufs=2, name=f"arep{hb}_{h2}")
                nc.sync.dma_start(a_rep[:], plb_sp.ap()[0, hsl].partition_broadcast(128))
                xa = ap.tile([128, HT], bf16, tag="pa", bufs=2, name=f"xa{hb}_{h2}")
                nc.vector.tensor_tensor(xa[:], xf[h2][hb][:], a_rep[:], ALU.mult)
                nc.vector.tensor_reduce(vv[:, h2 * BH:(h2 + 1) * BH],
                                        xa[:].rearrange("p (b t) -> p b t", b=BH),
                                        axis=AX.X, op=ALU.add)
            v16 = ap.tile([128, BS], bf16, tag="vshb", bufs=2, name=f"v16_{hb}")
            nc.vector.tensor_copy(v16[:], vv[:])
            v_t.append(v16)

        # ---------------- image branch + fusion head ----------------
        xiT = ap.tile([ID, BS], f32, tag="xiT", bufs=1, name="xiT")
        nc.sync.dma_start(xiT[:], xi_d.ap().rearrange("b f -> f b"))
        xiT16 = ap.tile([ID, BS], bf16, tag="xiT16", bufs=1, name="xiT16")
        nc.vector.tensor_copy(xiT16[:], xiT[:])
        ii1 = []
        for hb in range(HB):
            ps = pps.tile([128, BS], f32, tag="pj", name=f"i1p{hb}")
            nc.tensor.matmul(ps[:], imgw1T[0][0:ID, hb * 128:(hb + 1) * 128], xiT16[:],
                             start=True, stop=True)
            t = ap.tile([128, BS], bf16, tag="ii1t", bufs=2, name=f"ii1_{hb}")
            nc.scalar.activation(t[:], ps[:], AF.Relu, bias=imgb1[hb][:, 0:1])
            ii1.append(t)
        ii2 = []
        for hb in range(HB):
            ps = pps.tile([128, BS], f32, tag="pj", name=f"i2p{hb}")
            for kb in range(HB):
                nc.tensor.matmul(ps[:], imgw2T[kb][:, hb * 128:(hb + 1) * 128],
                                 ii1[kb][:], start=(kb == 0), stop=(kb == HB - 1))
            t = ap.tile([128, BS], bf16, tag="ii2t", bufs=2, name=f"ii2_{hb}")
            nc.scalar.activation(t[:], ps[:], AF.Relu, bias=imgb2[hb][:, 0:1])
            ii2.append(t)
        vi = []
        for hb in range(HB):
            t = ap.tile([128, BS], bf16, tag="vit", bufs=2, name=f"vi{hb}")
            nc.vector.tensor_tensor(t[:], v_t[hb][:], ii2[hb][:], ALU.mult)
            vi.append(t)
        f_rhs = [v_t[0], v_t[1], ii2[0], ii2[1], vi[0], vi[1]]
        hh = []
        for mb in range(HB):
            ps = pps.tile([128, BS], f32, tag="pj", name=f"h1p{mb}")
            for kb in range(6):
                nc.tensor.matmul(ps[:], h1T[kb][:, mb * 128:(mb + 1) * 128],
                                 f_rhs[kb][:], start=(kb == 0), stop=(kb == 5))
            t = ap.tile([128, BS], bf16, tag="hht", bufs=2, name=f"hh{mb}")
            nc.scalar.activation(t[:], ps[:], AF.Relu, bias=hb1[mb][:, 0:1])
            hh.append(t)
        ps = pps.tile([1, BS], f32, tag="pj", name="outp")
        for kb in range(HB):
            nc.tensor.matmul(ps[:], h2T[kb][:, 0:1], hh[kb][:],
                             start=(kb == 0), stop=(kb == HB - 1))
        o_sb = ap.tile([1, BS], f32, tag="osb", bufs=1, name="o_sb")
        nc.scalar.activation(o_sb[:], ps[:], AF.Identity, bias=hb2[0:1, 0:1])
        nc.sync.dma_start(out_d.ap(), o_sb[:])

    nc.compile()
    return nc


_NC = None


def _get_nc():
    global _NC
    if _NC is None:
        _NC = _build()
    return _NC


def run(inputs, trace=False):
    nc = _get_nc()
    inputs = {k: np.asarray(v, dtype=np.float32) for k, v in inputs.items()}
    in_maps = []
    for c in range(NCORES):
        m = {name: inputs[name] for name in WEIGHT_NAMES}
        m["xv"] = np.ascontiguousarray(inputs["xv"][c * BS:(c + 1) * BS])
        m["xi"] = np.ascontiguousarray(inputs["xi"][c * BS:(c + 1) * BS])
        in_maps.append(m)
    res = run_bass_kernel_spmd(nc, in_maps, core_ids=list(range(NCORES)), trace=trace)
    out = np.concatenate([np.asarray(res.results[c]["out"]).reshape(BS)
                          for c in range(NCORES)])
    return out.reshape(B, 1).astype(np.float32), res.exec_time_ns


def kernel(**inputs):
    return run(inputs, trace=False)[0]
